# revision 10
# baseline (speedup 1.0000x reference)
"""Causal attention kernel for Trainium2 (Bass/Tile), SPMD over 8 NeuronCores.

Problem: B=16, N=2048, D=256 fp32 causal attention with padding mask.
Sharding: batch dim across 8 cores (2 batches per core); attention is
batch-independent so no collectives are needed. The host wrapper passes
Q^T/K^T (B, D, N) slices in bf16 so the device streams them straight into
the d-on-partitions layout the PE needs, and the padding mask pre-baked
as an additive bias in [128, 16] per-partition layout.

Per-core algorithm (S^T orientation: k on partitions, q on free axis):
  S^T = K @ Q^T computed chunkwise as (K^T chunk).T @ Q^T  [bf16 matmuls]
  P^T = exp(scale * S^T + pad_bias)   [ScalarE, pad bias is per-partition]
  [O | rowsum] = P @ [V | 1]          [ones-column gives softmax denominators]
  O = O * (1/rowsum)                  [bf16 out, host upcasts to f32]
"""

import numpy as np

import concourse.bass as bass
from concourse import bacc
import concourse.mybir as mybir
from concourse import tile
from concourse.bass_utils import run_bass_kernel_spmd

F32 = mybir.dt.float32
I32 = mybir.dt.int32
BF16 = mybir.dt.bfloat16

N_CORES = 8
B_FULL, N_SEQ, D_MODEL = 16, 2048, 256
B_LOCAL = B_FULL // N_CORES

NEG = -1e30
P = 128
VW = 2  # extra cols on V: [ones, zero-pad]


def build_attention_nc(B=B_LOCAL, N=N_SEQ, D=D_MODEL, QBS=512):
    nc = bacc.Bacc(num_swdge_queues=4)
    NT = N // P            # number of 128-row tiles along sequence
    DC = D // P            # number of 128-wide d chunks
    TB = QBS // P          # q tiles per q block
    NB = N // QBS          # number of q blocks
    scale = 1.0 / float(np.sqrt(D))

    qt_d = nc.declare_dram_parameter("qt", [B, D, N], BF16, isOutput=False)
    kt_d = nc.declare_dram_parameter("kt", [B, D, N], BF16, isOutput=False)
    v_d = nc.declare_dram_parameter("v", [B, N, D], BF16, isOutput=False)
    pb_d = nc.declare_dram_parameter("pb", [B, P, NT], F32, isOutput=False)
    o_d = nc.declare_dram_parameter("o", [B, N, D], BF16, isOutput=True)

    with tile.TileContext(nc) as tc:
        with (
            tc.tile_pool(name="consts", bufs=1) as consts,
            tc.tile_pool(name="big", bufs=2) as big,
            tc.tile_pool(name="ptp", bufs=6) as ptp,
            tc.tile_pool(name="smallp", bufs=4) as smallp,
            tc.tile_pool(name="ps_sp", bufs=4, space="PSUM") as ps_sp,
            tc.tile_pool(name="ps_op", bufs=TB, space="PSUM") as ps_op,
        ):
            # Additive causal mask for the diagonal 128x128 chunk of S^T:
            # element [k_local, q_local] valid iff k <= q, i.e. keep where
            # (q - k) >= 0, else fill with NEG.
            dmask = consts.tile([P, P], F32)
            nc.gpsimd.memset(dmask, 0.0)
            nc.gpsimd.affine_select(
                out=dmask,
                in_=dmask,
                compare_op=mybir.AluOpType.is_ge,
                fill=NEG,
                base=0,
                pattern=[[1, P]],
                channel_multiplier=-1,
            )

            # Prime the ScalarE exp table so the 1.5us ACT_TABLE_LOAD isn't
            # on the first real exp's critical path.
            warm_in = consts.tile([P, 1], F32)
            warm_out = consts.tile([P, 1], BF16)
            nc.vector.memset(warm_in, 0.0)
            nc.scalar.activation(
                warm_out, warm_in, mybir.ActivationFunctionType.Exp,
                bias=0.0, scale=1.0,
            )

            for b in range(B):
                # ---- per-batch loads ----
                kT = big.tile([P, DC, N], BF16, tag="kT")
                qT = big.tile([P, DC, N], BF16, tag="qT")
                vx = big.tile([P, NT, D + VW], BF16, tag="vx")
                ostg = big.tile([P, NT, D], BF16, tag="ostg")
                pbias = big.tile([P, NT], F32, tag="pbias")

                qt_r = qt_d[b].rearrange("(dc p) n -> p dc n", p=P)
                kt_r = kt_d[b].rearrange("(dc p) n -> p dc n", p=P)
                v_r = v_d[b].rearrange("(c p) d -> p c d", p=P)

                # Sync queue carries Q; GpSimd carries the first K chunks, the
                # padding bias and V — so the first QK^T's operands (kT dc0
                # cols 0:128 and qT cols 0:512) land in parallel ASAP.
                # Three DMA issuers in parallel: Sync carries Q, Scalar (the
                # second HWDGE engine, idle until the first exp) carries K,
                # GpSimd carries the first K tile + padding bias + V.
                h0 = QBS // 2
                nc.sync.dma_start(out=qT[:, :, 0:h0], in_=qt_r[:, :, 0:h0])
                nc.sync.dma_start(out=qT[:, :, h0:QBS], in_=qt_r[:, :, h0:QBS])
                nc.gpsimd.dma_start(out=kT[:, 0, 0:128], in_=kt_r[:, 0, 0:128])
                nc.gpsimd.dma_start(out=kT[:, 1, 0:128], in_=kt_r[:, 1, 0:128])
                nc.gpsimd.dma_start(out=pbias, in_=pb_d[b])
                nc.scalar.dma_start(out=kT[:, :, 128:512], in_=kt_r[:, :, 128:512])
                nc.sync.dma_start(out=qT[:, :, 512:1024], in_=qt_r[:, :, 512:1024])
                nc.scalar.dma_start(out=kT[:, :, 512:1024], in_=kt_r[:, :, 512:1024])
                nc.sync.dma_start(out=qT[:, :, 1024:1536], in_=qt_r[:, :, 1024:1536])
                nc.scalar.dma_start(out=kT[:, :, 1024:1536], in_=kt_r[:, :, 1024:1536])
                nc.sync.dma_start(out=qT[:, :, 1536:2048], in_=qt_r[:, :, 1536:2048])
                nc.scalar.dma_start(out=kT[:, :, 1536:2048], in_=kt_r[:, :, 1536:2048])
                G = 4  # V tiles per DMA group
                for g0 in range(0, NT, G):
                    nc.gpsimd.dma_start(
                        out=vx[:, g0 : g0 + G, 0:D], in_=v_r[:, g0 : g0 + G, :]
                    )

                # ones column at D (softmax denominator trick); D+1 zero pad
                # keeps the PV moving operand width even (258).
                nc.vector.memset(vx[:, :, D : D + VW], 0.0)
                nc.vector.memset(vx[:, :, D : D + 1], 1.0)

                # ---- main attention loop over q blocks ----
                o_r = o_d[b].rearrange("(c p) d -> p c d", p=P)
                for qb in range(NB):
                    tbase = qb * TB
                    po = [ps_op.tile([P, D + VW], F32, tag="po", name=f"po{i}")
                          for i in range(TB)]
                    for j in range(tbase + TB):
                        ls = max(0, (j - tbase) * P)
                        ss = ps_sp.tile([P, QBS], F32, tag="ss")
                        if b == 0 and qb == 0 and j == 0:
                            # split the very first matmul in halves so it can
                            # start as soon as the first half of qT lands
                            for hh in range(2):
                                hs = slice(hh * (QBS // 2), (hh + 1) * (QBS // 2))
                                for dc in range(DC):
                                    nc.tensor.matmul(
                                        ss[:, hs],
                                        kT[:, dc, 0:P],
                                        qT[:, dc, hs],
                                        start=(dc == 0),
                                        stop=(dc == DC - 1),
                                    )
                        else:
                            for dc in range(DC):
                                nc.tensor.matmul(
                                    ss[:, ls:QBS],
                                    kT[:, dc, j * P : (j + 1) * P],
                                    qT[:, dc, qb * QBS + ls : (qb + 1) * QBS],
                                    start=(dc == 0),
                                    stop=(dc == DC - 1),
                                )
                        if j >= tbase:
                            nc.vector.tensor_add(
                                ss[:, ls : ls + P],
                                ss[:, ls : ls + P],
                                dmask,
                            )
                        pt = ptp.tile([P, QBS], BF16, tag="pt")
                        nc.scalar.activation(
                            pt[:, ls:QBS],
                            ss[:, ls:QBS],
                            mybir.ActivationFunctionType.Exp,
                            bias=pbias[:, j : j + 1],
                            scale=scale,
                        )
                        for ti in range(TB):
                            t = tbase + ti
                            if j <= t:
                                nc.tensor.matmul(
                                    po[ti],
                                    pt[:, ti * P : (ti + 1) * P],
                                    vx[:, j, 0 : D + VW],
                                    start=(j == 0),
                                    stop=(j == t),
                                )
                    last_block = b == B - 1 and qb == NB - 1
                    for ti in range(TB):
                        t = tbase + ti
                        rec = smallp.tile([P, 1], F32, tag="rec")
                        nc.vector.reciprocal(rec, po[ti][:, D : D + 1])
                        if last_block and ti == TB - 1:
                            # split the very last tile so its two output DMAs
                            # can issue on different queues in parallel
                            h = D // 2
                            nc.vector.tensor_scalar_mul(
                                ostg[:, t, 0:h], po[ti][:, 0:h], rec
                            )
                            nc.gpsimd.dma_start(
                                out=o_r[:, t : t + 1, 0:h],
                                in_=ostg[:, t : t + 1, 0:h],
                            )
                            nc.vector.tensor_scalar_mul(
                                ostg[:, t, h:D], po[ti][:, h:D], rec
                            )
                            nc.sync.dma_start(
                                out=o_r[:, t : t + 1, h:D],
                                in_=ostg[:, t : t + 1, h:D],
                            )
                        else:
                            nc.vector.tensor_scalar_mul(
                                ostg[:, t, :], po[ti][:, 0:D], rec
                            )
                            if last_block and ti == TB - 2:
                                nc.sync.dma_start(
                                    out=o_r[:, t : t + 1, :],
                                    in_=ostg[:, t : t + 1, :],
                                )
                    if last_block:
                        nc.gpsimd.dma_start(
                            out=o_r[:, tbase : tbase + TB - 2, :],
                            in_=ostg[:, tbase : tbase + TB - 2, :],
                        )
                    else:
                        nc.gpsimd.dma_start(
                            out=o_r[:, tbase : tbase + TB, :],
                            in_=ostg[:, tbase : tbase + TB, :],
                        )

    nc.finalize()
    return nc


_NC_CACHE = {}


def _get_nc():
    key = (B_LOCAL, N_SEQ, D_MODEL)
    if key not in _NC_CACHE:
        _NC_CACHE[key] = build_attention_nc()
    return _NC_CACHE[key]


def _make_in_maps(Q, K, V, padding_mask):
    import ml_dtypes

    bf16 = ml_dtypes.bfloat16
    QT = np.ascontiguousarray(
        np.asarray(Q, dtype=np.float32).transpose(0, 2, 1).astype(bf16)
    )
    KT = np.ascontiguousarray(
        np.asarray(K, dtype=np.float32).transpose(0, 2, 1).astype(bf16)
    )
    Vb = np.ascontiguousarray(np.asarray(V, dtype=np.float32).astype(bf16))
    pm = np.asarray(padding_mask)
    # additive bias: 0 where mask!=0, -1e30 where 0; [B, N] -> [B, 128, 16]
    # so partition p, col c holds bias for key index c*128+p.
    pb = np.where(pm != 0, 0.0, NEG).astype(np.float32)
    pb = np.ascontiguousarray(
        pb.reshape(B_FULL, N_SEQ // P, P).transpose(0, 2, 1)
    )

    in_maps = []
    for c in range(N_CORES):
        s = slice(c * B_LOCAL, (c + 1) * B_LOCAL)
        in_maps.append({"qt": QT[s], "kt": KT[s], "v": Vb[s], "pb": pb[s]})
    return in_maps


def kernel(Q, K, V, padding_mask):
    nc = _get_nc()
    in_maps = _make_in_maps(Q, K, V, padding_mask)
    res = run_bass_kernel_spmd(nc, in_maps, list(range(N_CORES)))
    out = np.concatenate([res.results[c]["o"] for c in range(N_CORES)], axis=0)
    return out.astype(np.float32)


# revision 11
# speedup vs baseline: 1.0313x; 1.0313x over previous
"""Causal attention kernel for Trainium2 (Bass/Tile), SPMD over 8 NeuronCores.

Problem: B=16, N=2048, D=256 fp32 causal attention with padding mask.
Sharding: batch dim across 8 cores (2 batches per core); attention is
batch-independent so no collectives are needed. The host wrapper passes
Q^T/K^T (B, D, N) slices in bf16 so the device streams them straight into
the d-on-partitions layout the PE needs, and the padding mask pre-baked
as an additive bias in [128, 16] per-partition layout.

Per-core algorithm (S^T orientation: k on partitions, q on free axis):
  S^T = K @ Q^T computed chunkwise as (K^T chunk).T @ Q^T  [bf16 matmuls]
  P^T = exp(scale * S^T + pad_bias)   [ScalarE, pad bias is per-partition]
  [O | rowsum] = P @ [V | 1]          [ones-column gives softmax denominators]
  O = O * (1/rowsum)                  [bf16 out, host upcasts to f32]
"""

import numpy as np

import concourse.bass as bass
from concourse import bacc
import concourse.mybir as mybir
from concourse import tile
from concourse.bass_utils import run_bass_kernel_spmd

F32 = mybir.dt.float32
I32 = mybir.dt.int32
BF16 = mybir.dt.bfloat16

N_CORES = 8
B_FULL, N_SEQ, D_MODEL = 16, 2048, 256
B_LOCAL = B_FULL // N_CORES

NEG = -1e30
P = 128
VW = 2  # extra cols on V: [ones, zero-pad]


def build_attention_nc(B=B_LOCAL, N=N_SEQ, D=D_MODEL, QBS=512):
    nc = bacc.Bacc(num_swdge_queues=4)
    NT = N // P            # number of 128-row tiles along sequence
    DC = D // P            # number of 128-wide d chunks
    TB = QBS // P          # q tiles per q block
    NB = N // QBS          # number of q blocks
    scale = 1.0 / float(np.sqrt(D))

    qt_d = nc.declare_dram_parameter("qt", [B, D, N], BF16, isOutput=False)
    kt_d = nc.declare_dram_parameter("kt", [B, D, N], BF16, isOutput=False)
    v_d = nc.declare_dram_parameter("v", [B, N, D], BF16, isOutput=False)
    pb_d = nc.declare_dram_parameter("pb", [B, P, NT], F32, isOutput=False)
    o_d = nc.declare_dram_parameter("o", [B, N, D], BF16, isOutput=True)

    with tile.TileContext(nc) as tc:
        with (
            tc.tile_pool(name="consts", bufs=1) as consts,
            tc.tile_pool(name="big", bufs=2) as big,
            tc.tile_pool(name="ptp", bufs=6) as ptp,
            tc.tile_pool(name="smallp", bufs=4) as smallp,
            tc.tile_pool(name="ps_sp", bufs=4, space="PSUM") as ps_sp,
            tc.tile_pool(name="ps_op", bufs=TB, space="PSUM") as ps_op,
        ):
            # Additive causal mask for the diagonal 128x128 chunk of S^T:
            # element [k_local, q_local] valid iff k <= q, i.e. keep where
            # (q - k) >= 0, else fill with NEG.
            dmask = consts.tile([P, P], F32)
            nc.gpsimd.memset(dmask, 0.0)
            nc.gpsimd.affine_select(
                out=dmask,
                in_=dmask,
                compare_op=mybir.AluOpType.is_ge,
                fill=NEG,
                base=0,
                pattern=[[1, P]],
                channel_multiplier=-1,
            )

            # Prime the ScalarE exp table so the 1.5us ACT_TABLE_LOAD isn't
            # on the first real exp's critical path.
            warm_in = consts.tile([P, 1], F32)
            warm_out = consts.tile([P, 1], BF16)
            nc.vector.memset(warm_in, 0.0)
            nc.scalar.activation(
                warm_out, warm_in, mybir.ActivationFunctionType.Exp,
                bias=0.0, scale=1.0,
            )

            for b in range(B):
                # ---- per-batch loads ----
                kT = big.tile([P, DC, N], BF16, tag="kT")
                qT = big.tile([P, DC, N], BF16, tag="qT")
                vx = big.tile([P, NT, D + VW], BF16, tag="vx")
                ostg = big.tile([P, NT, D], BF16, tag="ostg")
                pbias = big.tile([P, NT], F32, tag="pbias")

                qt_r = qt_d[b].rearrange("(dc p) n -> p dc n", p=P)
                kt_r = kt_d[b].rearrange("(dc p) n -> p dc n", p=P)
                v_r = v_d[b].rearrange("(c p) d -> p c d", p=P)

                # Sync queue carries Q; GpSimd carries the first K chunks, the
                # padding bias and V — so the first QK^T's operands (kT dc0
                # cols 0:128 and qT cols 0:512) land in parallel ASAP.
                # Sync queue carries Q (+ later K chunks); GpSimd carries the
                # first K tile, the padding bias and V — so the first QK^T's
                # operands land in parallel ASAP.
                h0 = QBS // 2
                nc.sync.dma_start(out=qT[:, :, 0:h0], in_=qt_r[:, :, 0:h0])
                nc.sync.dma_start(out=qT[:, :, h0:QBS], in_=qt_r[:, :, h0:QBS])
                nc.gpsimd.dma_start(out=kT[:, 0, 0:128], in_=kt_r[:, 0, 0:128])
                nc.gpsimd.dma_start(out=kT[:, 1, 0:128], in_=kt_r[:, 1, 0:128])
                nc.gpsimd.dma_start(out=pbias, in_=pb_d[b])
                nc.sync.dma_start(out=kT[:, :, 128:512], in_=kt_r[:, :, 128:512])
                G = 4  # V tiles per DMA group
                nc.gpsimd.dma_start(out=vx[:, 0:G, 0:D], in_=v_r[:, 0:G, :])
                for h in range(1, N // 512):
                    sl = slice(h * 512, (h + 1) * 512)
                    # q block h is needed a full round before kT cols h*512+
                    nc.sync.dma_start(out=qT[:, :, sl], in_=qt_r[:, :, sl])
                    nc.sync.dma_start(out=kT[:, :, sl], in_=kt_r[:, :, sl])
                    g0 = h * G
                    nc.gpsimd.dma_start(
                        out=vx[:, g0 : g0 + G, 0:D], in_=v_r[:, g0 : g0 + G, :]
                    )

                # ones column at D (softmax denominator trick); D+1 zero pad
                # keeps the PV moving operand width even (258).
                nc.vector.memset(vx[:, :, D : D + VW], 0.0)
                nc.vector.memset(vx[:, :, D : D + 1], 1.0)

                # ---- main attention loop over q blocks ----
                o_r = o_d[b].rearrange("(c p) d -> p c d", p=P)
                for qb in range(NB):
                    tbase = qb * TB
                    po = [ps_op.tile([P, D + VW], F32, tag="po", name=f"po{i}")
                          for i in range(TB)]
                    for j in range(tbase + TB):
                        ls = max(0, (j - tbase) * P)
                        ss = ps_sp.tile([P, QBS], F32, tag="ss")
                        if b == 0 and qb == 0 and j == 0:
                            # split the very first matmul in halves so it can
                            # start as soon as the first half of qT lands
                            for hh in range(2):
                                hs = slice(hh * (QBS // 2), (hh + 1) * (QBS // 2))
                                for dc in range(DC):
                                    nc.tensor.matmul(
                                        ss[:, hs],
                                        kT[:, dc, 0:P],
                                        qT[:, dc, hs],
                                        start=(dc == 0),
                                        stop=(dc == DC - 1),
                                    )
                        else:
                            for dc in range(DC):
                                nc.tensor.matmul(
                                    ss[:, ls:QBS],
                                    kT[:, dc, j * P : (j + 1) * P],
                                    qT[:, dc, qb * QBS + ls : (qb + 1) * QBS],
                                    start=(dc == 0),
                                    stop=(dc == DC - 1),
                                )
                        if j >= tbase:
                            nc.vector.tensor_add(
                                ss[:, ls : ls + P],
                                ss[:, ls : ls + P],
                                dmask,
                            )
                        pt = ptp.tile([P, QBS], BF16, tag="pt")
                        nc.scalar.activation(
                            pt[:, ls:QBS],
                            ss[:, ls:QBS],
                            mybir.ActivationFunctionType.Exp,
                            bias=pbias[:, j : j + 1],
                            scale=scale,
                        )
                        for ti in range(TB):
                            t = tbase + ti
                            if j <= t:
                                nc.tensor.matmul(
                                    po[ti],
                                    pt[:, ti * P : (ti + 1) * P],
                                    vx[:, j, 0 : D + VW],
                                    start=(j == 0),
                                    stop=(j == t),
                                )
                    last_block = b == B - 1 and qb == NB - 1
                    for ti in range(TB):
                        t = tbase + ti
                        rec = smallp.tile([P, 1], F32, tag="rec")
                        nc.vector.reciprocal(rec, po[ti][:, D : D + 1])
                        if last_block and ti == TB - 1:
                            # split the very last tile so its two output DMAs
                            # can issue on different queues in parallel
                            h = D // 2
                            nc.vector.tensor_scalar_mul(
                                ostg[:, t, 0:h], po[ti][:, 0:h], rec
                            )
                            nc.gpsimd.dma_start(
                                out=o_r[:, t : t + 1, 0:h],
                                in_=ostg[:, t : t + 1, 0:h],
                            )
                            nc.vector.tensor_scalar_mul(
                                ostg[:, t, h:D], po[ti][:, h:D], rec
                            )
                            nc.sync.dma_start(
                                out=o_r[:, t : t + 1, h:D],
                                in_=ostg[:, t : t + 1, h:D],
                            )
                        else:
                            nc.vector.tensor_scalar_mul(
                                ostg[:, t, :], po[ti][:, 0:D], rec
                            )
                            if last_block and ti == TB - 2:
                                nc.sync.dma_start(
                                    out=o_r[:, t : t + 1, :],
                                    in_=ostg[:, t : t + 1, :],
                                )
                    if last_block:
                        nc.gpsimd.dma_start(
                            out=o_r[:, tbase : tbase + TB - 2, :],
                            in_=ostg[:, tbase : tbase + TB - 2, :],
                        )
                    else:
                        nc.gpsimd.dma_start(
                            out=o_r[:, tbase : tbase + TB, :],
                            in_=ostg[:, tbase : tbase + TB, :],
                        )

    nc.finalize()
    return nc


_NC_CACHE = {}


def _get_nc():
    key = (B_LOCAL, N_SEQ, D_MODEL)
    if key not in _NC_CACHE:
        _NC_CACHE[key] = build_attention_nc()
    return _NC_CACHE[key]


def _make_in_maps(Q, K, V, padding_mask):
    import ml_dtypes

    bf16 = ml_dtypes.bfloat16
    QT = np.ascontiguousarray(
        np.asarray(Q, dtype=np.float32).transpose(0, 2, 1).astype(bf16)
    )
    KT = np.ascontiguousarray(
        np.asarray(K, dtype=np.float32).transpose(0, 2, 1).astype(bf16)
    )
    Vb = np.ascontiguousarray(np.asarray(V, dtype=np.float32).astype(bf16))
    pm = np.asarray(padding_mask)
    # additive bias: 0 where mask!=0, -1e30 where 0; [B, N] -> [B, 128, 16]
    # so partition p, col c holds bias for key index c*128+p.
    pb = np.where(pm != 0, 0.0, NEG).astype(np.float32)
    pb = np.ascontiguousarray(
        pb.reshape(B_FULL, N_SEQ // P, P).transpose(0, 2, 1)
    )

    in_maps = []
    for c in range(N_CORES):
        s = slice(c * B_LOCAL, (c + 1) * B_LOCAL)
        in_maps.append({"qt": QT[s], "kt": KT[s], "v": Vb[s], "pb": pb[s]})
    return in_maps


def kernel(Q, K, V, padding_mask):
    nc = _get_nc()
    in_maps = _make_in_maps(Q, K, V, padding_mask)
    res = run_bass_kernel_spmd(nc, in_maps, list(range(N_CORES)))
    out = np.concatenate([res.results[c]["o"] for c in range(N_CORES)], axis=0)
    return out.astype(np.float32)
